# revision 4
# baseline (speedup 1.0000x reference)
"""v2 distributed GAT kernel for 8 TRN2 NeuronCores (Bass/Tile).

Structure vs v1: quarter tables keyed by slot-range (so each of the 4
per-layer AllGathers fires as soon as its quarter of the slab is
written), range-chunked gathers (~52/layer instead of 196), self-loop
contributions folded locally from SBUF stashes (no gather slots),
in-place bf16 edge weighting, per-group batched row DMA.
"""
import sys
import numpy as np

for _p in ('/opt/trn_rl_repo', '/root/.axon_site/_ro/trn_rl_repo'):
    if _p not in sys.path:
        sys.path.append(_p)

import concourse.bacc as bacc
from concourse import mybir, masks
from concourse.tile import TileContext
from concourse.bass_utils import run_bass_kernel_spmd
from contextlib import ExitStack

NCORES, NQ = 8, 4
BLOCKS, SPQ = 100, 25
SLAB = BLOCKS * 128
QR = SPQ * 128              # 3200 slab rows per quarter
TQ = NCORES * QR            # 25600 table rows per quarter
GS = 2
NGRAPHS = 256
GPC = NGRAPHS // NCORES
NL = 4
NEG, NEG_OUT = 0.2, 0.01
HEADS = (4, 4, 4, 1)

FP = mybir.dt.float32
BF = mybir.dt.bfloat16
I16 = mybir.dt.int16
ALU = mybir.AluOpType
ACTF = mybir.ActivationFunctionType
AX = mybir.AxisListType


# --------------------------------------------------------------- preprocess
def preprocess(edge_index, batch, col_cap=30):
    N = 100000
    src = np.asarray(edge_index[0], dtype=np.int64)
    dst = np.asarray(edge_index[1], dtype=np.int64)
    batch = np.asarray(batch, dtype=np.int64)
    E = src.shape[0]

    outdeg = np.bincount(src, minlength=N)

    o = np.argsort(-outdeg, kind='stable')
    quarter = np.empty(N, np.int64)
    s_, j_ = np.divmod(np.arange(N), NQ)
    quarter[o] = np.where((s_ % 2) == 0, j_, NQ - 1 - j_)

    degq = np.zeros((N, NQ), np.int64)
    np.add.at(degq, (dst, quarter[src]), 1)

    node_core = np.empty(N, np.int64)
    node_slot = np.empty(N, np.int64)
    node_row = np.empty(N, np.int64)
    for p in range(NQ):
        nodes = np.where(quarter == p)[0]
        d = degq[nodes]
        mx = d.max(axis=1)
        order = np.argsort(-mx, kind='stable')
        W = np.zeros((SPQ, NQ), np.int64)
        cnt = np.zeros(SPQ, np.int64)
        cap = np.full(SPQ, 1024)
        cap[SPQ - 1] -= 8
        slot_of = np.empty(len(nodes), np.int64)
        for i in order:
            di = d[i]
            inc = np.maximum(di[None, :] - W, 0).sum(axis=1).astype(np.float64)
            inc += 1e-6 * (cap - cnt)
            inc[cnt >= cap] = 1e18
            s = int(np.argmin(inc))
            slot_of[i] = s
            np.maximum(W[s], di, out=W[s])
            cnt[s] += 1
        for s in range(SPQ):
            sel = nodes[order[slot_of[order] == s]]
            r = np.arange(len(sel))
            node_core[sel] = r % NCORES
            node_slot[sel] = p * SPQ + s
            node_row[sel] = r // NCORES
    tpos = node_core * SLAB + node_slot * 128 + node_row

    groups = []
    for p in range(NQ):
        s = 0
        while s < SPQ:
            nsl = min(GS, SPQ - s)
            groups.append((p * SPQ + s, nsl, p))
            s += nsl
    NGRP = len(groups)
    grp_of_slot = np.empty(BLOCKS, np.int64)
    for gi, (lo, nsl, _) in enumerate(groups):
        grp_of_slot[lo:lo + nsl] = gi

    cntc = np.zeros((NCORES, BLOCKS, 128, NQ), np.int64)
    np.add.at(cntc, (node_core[dst], node_slot[dst], node_row[dst],
                     quarter[src]), 1)
    slotq_max = cntc.max(axis=(0, 2))
    WG = np.zeros((NGRP, NQ), np.int64)
    for gi, (lo, nsl, _) in enumerate(groups):
        WG[gi] = np.maximum(slotq_max[lo:lo + nsl].max(axis=0), 1)

    nsl_arr = np.array([g[1] for g in groups])
    ranges = []
    g = 0
    while g < NGRP:
        h = g + 1
        while h < NGRP and int((WG[g:h + 1] * nsl_arr[g:h + 1, None])
                               .sum(axis=0).max()) <= col_cap:
            h += 1
        ranges.append((g, h))
        g = h
    NRNG = len(ranges)

    col0 = np.zeros((NQ, NGRP), np.int64)
    chunk_cols = np.zeros((NQ, NRNG), np.int64)
    chunk_base = np.zeros((NQ, NRNG + 1), np.int64)
    for q in range(NQ):
        c = 0
        for ri, (glo, ghi) in enumerate(ranges):
            chunk_base[q, ri] = c
            for gi in range(glo, ghi):
                col0[q, gi] = c
                c += groups[gi][1] * WG[gi, q]
            chunk_cols[q, ri] = c - chunk_base[q, ri]
        chunk_base[q, NRNG] = c
    QCOLS = chunk_base[:, NRNG].copy()

    sq = quarter[src]
    kk = (tpos[dst] * NQ) + sq
    eorder = np.argsort(kk, kind='stable')
    ks = kk[eorder]
    first = np.concatenate([[True], ks[1:] != ks[:-1]])
    runstart = np.maximum.accumulate(np.where(first, np.arange(E), 0))
    rank = np.arange(E) - runstart

    src_local = (node_core[src] * QR + (node_slot[src] - sq * SPQ) * 128
                 + node_row[src])
    assert src_local.max() < TQ

    lidx = [np.full((NCORES, 128, int(QCOLS[q])), -1, np.int64)
            for q in range(NQ)]
    eo = eorder
    d_core = node_core[dst][eo]
    d_row = node_row[dst][eo]
    d_slot = node_slot[dst][eo]
    gi_e = grp_of_slot[d_slot]
    q_e = sq[eo]
    glo_arr = np.array([g[0] for g in groups])
    sl_in_g = d_slot - glo_arr[gi_e]
    col = col0[q_e, gi_e] + sl_in_g * WG[gi_e, q_e] + rank
    for q in range(NQ):
        m = q_e == q
        lidx[q][d_core[m], d_row[m], col[m]] = src_local[eo[m]]
    for q in range(NQ):
        for c in range(NCORES):
            m = lidx[q][c] < 0
            lidx[q][c][m] = c * QR + 24 * 128 + 127

    # pooling
    gsize = np.bincount(batch, minlength=NGRAPHS)
    gorder = np.argsort(-gsize, kind='stable')
    pool_core = np.empty(NGRAPHS, np.int64)
    pool_row = np.empty(NGRAPHS, np.int64)
    for i, gg in enumerate(gorder):
        r, j = divmod(i, NCORES)
        pool_core[gg] = j if r % 2 == 0 else NCORES - 1 - j
        pool_row[gg] = r
    keyp = (pool_core[batch] * GPC + pool_row[batch]) * NQ + quarter
    porder = np.argsort(keyp, kind='stable')
    kp = keyp[porder]
    firstp = np.concatenate([[True], kp[1:] != kp[:-1]])
    runstart = np.maximum.accumulate(np.where(firstp, np.arange(N), 0))
    rankp = np.arange(N) - runstart
    subrow = rankp % 4
    jcol = rankp // 4
    pq = np.zeros((NCORES, GPC, NQ), np.int64)
    np.add.at(pq, (pool_core[batch], pool_row[batch], quarter), 1)
    PWQS = np.maximum((pq + 3) // 4, 1).max(axis=(0, 1))
    pq0 = np.concatenate([[0], np.cumsum(PWQS)])
    WPS = int(pq0[-1])
    node_localq = node_core * QR + (node_slot - quarter * SPQ) * 128 + node_row
    pool_lidx = np.zeros((NCORES, 128, WPS), np.int64)
    pool_pad = np.ones((NCORES, 128, WPS), bool)
    pc = pool_core[batch][porder]
    pr = pool_row[batch][porder]
    ppart = pr * 4 + subrow
    pcol = pq0[quarter[porder]] + jcol
    pool_lidx[pc, ppart, pcol] = node_localq[porder]
    pool_pad[pc, ppart, pcol] = False
    pool_maskneg = np.where(pool_pad, np.float32(-1e30), np.float32(0.0))
    out_graph = np.empty((NCORES, GPC), np.int64)
    out_graph[pool_core, pool_row] = np.arange(NGRAPHS)

    node_at = np.full((NCORES, SLAB), -1, np.int64)
    node_at[node_core, node_slot * 128 + node_row] = np.arange(N)

    return dict(
        groups=groups, WG=WG, ranges=ranges, col0=col0,
        chunk_cols=chunk_cols, chunk_base=chunk_base, QCOLS=QCOLS,
        lidx=lidx, node_at=node_at,
        pool_lidx=pool_lidx, pool_maskneg=pool_maskneg, PWQS=PWQS,
        pq0=pq0, WPS=WPS, out_graph=out_graph,
    )


def wrap_idx(vals):
    sh = vals.shape[:-1]
    n = vals.shape[-1]
    assert n % 16 == 0
    w = vals.reshape(*sh, n // 16, 16)
    w = np.swapaxes(w, -1, -2)
    w = np.broadcast_to(w[..., None, :, :], (*sh, 8, 16, n // 16))
    return w.reshape(*sh, 128, n // 16).copy()


def expand_a(a):
    heads, ch = a.shape
    A = np.zeros((heads * ch, heads), np.float32)
    for h in range(heads):
        A[h * ch:(h + 1) * ch, h] = a[h]
    return A


# --------------------------------------------------------------- build
def build(nc, geom):
    groups = geom['groups']
    WG = geom['WG']
    ranges = geom['ranges']
    col0 = geom['col0']
    chunk_cols = geom['chunk_cols']
    chunk_base = geom['chunk_base']
    QCOLS = geom['QCOLS']
    PWQS = geom['PWQS']
    pq0 = geom['pq0']
    WPS = int(geom['WPS'])
    NGRP = len(groups)
    NRNG = len(ranges)
    rng_of_grp = np.empty(NGRP, np.int64)
    for ri, (a, b) in enumerate(ranges):
        rng_of_grp[a:b] = ri

    xT = nc.declare_dram_parameter("xT", [128, SLAB], FP, isOutput=False)
    idx_in = [nc.declare_dram_parameter(f"idx{q}", [128, 8 * int(QCOLS[q])],
                                        I16, isOutput=False)
              for q in range(NQ)]
    pidx_in = nc.declare_dram_parameter("pool_idx", [128, 8 * WPS], I16,
                                        isOutput=False)
    pmask_in = nc.declare_dram_parameter("pool_maskneg", [128, WPS], FP,
                                         isOutput=False)
    wext_in = [nc.declare_dram_parameter(
        f"wext{l}", [128 if l == 0 else 64, 64 + 2 * HEADS[l]], FP,
        isOutput=False) for l in range(NL)]
    bias_in = nc.declare_dram_parameter("bias", [128, NL, 64], FP,
                                        isOutput=False)
    fcW_in = nc.declare_dram_parameter("fcW", [64, 2], FP, isOutput=False)
    fcb_in = nc.declare_dram_parameter("fcb", [GPC, 2], FP, isOutput=False)
    logits_out = nc.declare_dram_parameter("logits", [GPC, 2], FP,
                                           isOutput=True)
    probas_out = nc.declare_dram_parameter("probas", [GPC, 2], FP,
                                           isOutput=True)

    with TileContext(nc) as tc, ExitStack() as ex:
        dram = ex.enter_context(tc.tile_pool(name="dram", bufs=1, space="DRAM"))
        slabq = [[dram.tile([QR, 128], I16, name=f"slab{l}_{q}")
                  for q in range(NQ)] for l in range(NL + 1)]
        tabq = [[dram.tile([TQ, 128], I16, addr_space="Shared",
                           name=f"tab{l}_{q}")
                 for q in range(NQ)] for l in range(NL + 1)]

        cpool = ex.enter_context(tc.tile_pool(name="const", bufs=1))
        gpools = [ex.enter_context(tc.tile_pool(name=f"g{q}", bufs=4))
                  for q in range(NQ)]
        ipool = ex.enter_context(tc.tile_pool(name="idxs", bufs=8))
        wpool = ex.enter_context(tc.tile_pool(name="wrk", bufs=4))
        spool = ex.enter_context(tc.tile_pool(name="sml", bufs=8))
        xpool = ex.enter_context(tc.tile_pool(name="xin", bufs=2))
        rpool = ex.enter_context(tc.tile_pool(name="rows", bufs=3))
        ppool = ex.enter_context(tc.tile_pool(name="psum", bufs=4,
                                              space="PSUM"))
        ppool2 = ex.enter_context(tc.tile_pool(name="psum2", bufs=2,
                                               space="PSUM"))

        wext = []
        for l in range(NL):
            t = cpool.tile([128 if l == 0 else 64, 64 + 2 * HEADS[l]], FP,
                           name=f"wext_sb{l}")
            nc.sync.dma_start(t[:], wext_in[l][:])
            wext.append(t)
        bias_sb = cpool.tile([128, NL, 64], FP)
        nc.sync.dma_start(bias_sb[:], bias_in[:])
        ident = cpool.tile([128, 128], FP)
        masks.make_identity(nc, ident[:])
        poison_row = cpool.tile([1, 128], I16)
        nc.vector.memset(poison_row[:].bitcast(BF)[:, 0:64], 0.0)
        nc.vector.memset(poison_row[:].bitcast(FP)[:, 32:36], -120.0)
        nc.vector.memset(poison_row[:].bitcast(FP)[:, 36:64], 0.0)
        # per-layer-parity stashes
        sdst_self = [cpool.tile([128, BLOCKS, 4], FP, name=f"sd{i}")
                     for i in range(2)]
        ssrc_self = [cpool.tile([128, BLOCKS, 4], FP, name=f"ss{i}")
                     for i in range(2)]
        hloc = [cpool.tile([128, BLOCKS, 64], BF, name=f"hl{i}")
                for i in range(2)]

        def matmul_block(l, s, lhsT_ap, row_ap, si):
            """matmul for slot s of layer l; writes row si of the group row
            tile (int16 [128, nsl, 128]) + stashes."""
            H = HEADS[l]
            pm = ppool.tile([128, 64 + 2 * H], FP, tag="mm", name=f"mm{l}_{s}")
            nc.tensor.matmul(pm[:], lhsT_ap, wext[l][:], start=True, stop=True)
            # bf16 feats -> row cols 0:64 ; also hloc stash
            nc.scalar.copy(row_ap.bitcast(BF)[:, si, 0:64], pm[:, 0:64])
            nc.scalar.copy(hloc[l % 2][:, s, :], pm[:, 0:64])
            rf = row_ap.bitcast(FP)
            nc.scalar.copy(rf[:, si, 32:32 + H], pm[:, 64:64 + H])
            nc.scalar.copy(ssrc_self[l % 2][:, s, 0:H],
                           pm[:, 64:64 + H])
            nc.scalar.copy(sdst_self[l % 2][:, s, 0:H],
                           pm[:, 64 + H:64 + 2 * H])

        def emit_row_dma(l, gi, row):
            lo, nsl, dq = groups[gi]
            for si in range(nsl):
                s = lo + si
                rs = (s - dq * SPQ) * 128
                if s % SPQ == SPQ - 1:
                    nc.sync.dma_start(slabq[l][dq][rs:rs + 127, :],
                                      row[0:127, si, :])
                    nc.sync.dma_start(slabq[l][dq][rs + 127:rs + 128, :],
                                      poison_row[:])
                else:
                    nc.sync.dma_start(slabq[l][dq][rs:rs + 128, :],
                                      row[:, si, :])

        # ---------------- layer-0 matmul phase ----------------
        for gi, (lo, nsl, dq) in enumerate(groups):
            row = rpool.tile([128, nsl, 128], I16, tag="row", name=f"r0_{gi}")
            for si in range(nsl):
                s = lo + si
                xt = xpool.tile([128, 128], FP, tag="xt", name=f"x0_{s}")
                nc.sync.dma_start(xt[:], xT[:, s * 128:(s + 1) * 128])
                matmul_block(0, s, xt[:], row[:], si)
            emit_row_dma(0, gi, row)

        # ---------------- layers ----------------
        def emit_gathers(l):
            """Pool-ordered gather emission with q3 delayed by 2 ranges.
            AG(l, q3) is emitted after the first two ranges' q0..2."""
            def one(ri, q):
                ccols = int(chunk_cols[q][ri])
                it = ipool.tile([128, 8 * ccols], I16, tag="idx",
                                name=f"i{l}_{ri}_{q}")
                nc.sync.dma_start(
                    it[:], idx_in[q][:, 8 * int(chunk_base[q][ri]):
                                     8 * int(chunk_base[q][ri] + ccols)])
                t = gpools[q].tile([128, ccols, 128], I16, tag=f"G{q}",
                                   name=f"G{l}_{ri}_{q}")
                nc.gpsimd.dma_gather(
                    t[:], tabq[l][q][:], it[:], 128 * ccols, 128 * ccols, 128,
                    single_packet=False, queue_num=q)
                return t
            tiles = [[None] * NQ for _ in range(NRNG)]
            for ri in range(min(2, NRNG)):
                for q in range(3):
                    tiles[ri][q] = one(ri, q)
            nc.gpsimd.collective_compute(
                "AllGather", ALU.bypass, ins=[slabq[l][3][:].opt()],
                outs=[tabq[l][3][:].opt()],
                replica_groups=[list(range(NCORES))])
            for ri in range(min(2, NRNG)):
                tiles[ri][3] = one(ri, 3)
            for ri in range(2, NRNG):
                for q in range(NQ):
                    tiles[ri][q] = one(ri, q)
            return tiles

        for l in range(NL):
            H = HEADS[l]
            ch = 64 // H
            for q in range(3):
                nc.gpsimd.collective_compute(
                    "AllGather", ALU.bypass, ins=[slabq[l][q][:].opt()],
                    outs=[tabq[l][q][:].opt()],
                    replica_groups=[list(range(NCORES))])
            tiles = emit_gathers(l)

            for gi, (lo, nsl, dq) in enumerate(groups):
                ri = int(rng_of_grp[gi])
                den = spool.tile([128, nsl, 4], FP, tag="den",
                                 name=f"den{l}_{gi}")
                outg = wpool.tile([128, nsl, 64], FP, tag="outg",
                                  name=f"og{l}_{gi}")
                # ---- 4 quarter passes (q0 writes den/outg directly) ----
                for q in range(NQ):
                    w = int(WG[gi][q])
                    a = int(col0[q][gi] - chunk_base[q][ri])
                    Gt = tiles[ri][q]
                    Gf = Gt[:].bitcast(FP)[:, a:a + nsl * w, :].rearrange(
                        "p (s j) e -> p s j e", s=nsl)
                    Gb = Gt[:].bitcast(BF)[:, a:a + nsl * w, 0:64].rearrange(
                        "p (s j) (h c) -> p s j h c", s=nsl, h=H)
                    e = wpool.tile([128, nsl, w, H], FP, tag="e",
                                   name=f"e{l}_{gi}_{q}")
                    nc.vector.tensor_tensor(
                        e[:], Gf[:, :, :, 32:32 + H],
                        sdst_self[l % 2][:, lo:lo + nsl, 0:H]
                        .unsqueeze(2).broadcast_to([128, nsl, w, H]),
                        ALU.add)
                    nc.vector.scalar_tensor_tensor(e[:], e[:], NEG, e[:],
                                                   ALU.mult, ALU.max)
                    ext = wpool.tile([128, nsl, w, H], BF, tag="ex",
                                     name=f"ex{l}_{gi}_{q}")
                    nc.scalar.activation(ext[:], e[:], ACTF.Exp)
                    if q == 0:
                        nc.vector.tensor_reduce(
                            den[:, :, 0:H],
                            ext[:].rearrange("p s j h -> p s h j"),
                            axis=AX.X, op=ALU.add)
                    else:
                        dq_t = spool.tile([128, nsl, 4], FP, tag="dq",
                                          name=f"dq{l}_{gi}_{q}")
                        nc.vector.tensor_reduce(
                            dq_t[:, :, 0:H],
                            ext[:].rearrange("p s j h -> p s h j"),
                            axis=AX.X, op=ALU.add)
                        nc.vector.tensor_tensor(den[:, :, 0:H], den[:, :, 0:H],
                                                dq_t[:, :, 0:H], ALU.add)
                    # in-place bf16 weighting of gathered feats
                    nc.vector.tensor_tensor(
                        Gb, Gb,
                        ext[:].unsqueeze(4).broadcast_to([128, nsl, w, H, ch]),
                        ALU.mult)
                    if q == 0:
                        nc.vector.tensor_reduce(
                            outg[:], Gt[:].bitcast(BF)[:, a:a + nsl * w, 0:64]
                            .rearrange("p (s j) f -> p s f j", s=nsl),
                            axis=AX.X, op=ALU.add)
                    else:
                        wr = wpool.tile([128, nsl, 64], FP, tag="wr",
                                        name=f"wr{l}_{gi}_{q}")
                        nc.vector.tensor_reduce(
                            wr[:], Gt[:].bitcast(BF)[:, a:a + nsl * w, 0:64]
                            .rearrange("p (s j) f -> p s f j", s=nsl),
                            axis=AX.X, op=ALU.add)
                        nc.vector.tensor_tensor(outg[:], outg[:], wr[:],
                                                ALU.add)
                # ---- self contribution (after quarter passes) ----
                e_s = spool.tile([128, nsl, 4], FP, tag="es", name=f"es{l}_{gi}")
                nc.vector.tensor_tensor(e_s[:, :, 0:H],
                                        ssrc_self[l % 2][:, lo:lo + nsl, 0:H],
                                        sdst_self[l % 2][:, lo:lo + nsl, 0:H],
                                        ALU.add)
                nc.vector.scalar_tensor_tensor(e_s[:, :, 0:H], e_s[:, :, 0:H],
                                               NEG, e_s[:, :, 0:H],
                                               ALU.mult, ALU.max)
                ext_s = spool.tile([128, nsl, 4], FP, tag="exs",
                                   name=f"exs{l}_{gi}")
                nc.scalar.activation(ext_s[:, :, 0:H], e_s[:, :, 0:H], ACTF.Exp)
                nc.vector.tensor_tensor(den[:, :, 0:H], den[:, :, 0:H],
                                        ext_s[:, :, 0:H], ALU.add)
                tmp_s = wpool.tile([128, nsl, 64], FP, tag="wr",
                                   name=f"ts{l}_{gi}")
                nc.vector.tensor_tensor(
                    tmp_s[:].rearrange("p s (h c) -> p s h c", h=H),
                    hloc[l % 2][:, lo:lo + nsl, :]
                    .rearrange("p s (h c) -> p s h c", h=H),
                    ext_s[:, :, 0:H].unsqueeze(3)
                    .broadcast_to([128, nsl, H, ch]),
                    ALU.mult)
                nc.vector.tensor_tensor(outg[:], outg[:], tmp_s[:], ALU.add)
                # ---- finalize ----
                rden = spool.tile([128, nsl, 4], FP, tag="rd",
                                  name=f"rd{l}_{gi}")
                nc.vector.reciprocal(rden[:, :, 0:H], den[:, :, 0:H])
                nc.vector.tensor_tensor(
                    outg[:].rearrange("p s (h c) -> p s h c", h=H),
                    outg[:].rearrange("p s (h c) -> p s h c", h=H),
                    rden[:, :, 0:H].unsqueeze(3).broadcast_to(
                        [128, nsl, H, ch]),
                    ALU.mult)
                nc.vector.tensor_tensor(
                    outg[:], outg[:],
                    bias_sb[:, l, :].unsqueeze(1).broadcast_to([128, nsl, 64]),
                    ALU.add)
                nc.vector.scalar_tensor_tensor(outg[:], outg[:], NEG_OUT,
                                               outg[:], ALU.mult, ALU.max)
                # ---- next layer input / final rows ----
                row = rpool.tile([128, nsl, 128], I16, tag="row",
                                 name=f"r{l+1}_{gi}")
                for si in range(nsl):
                    s = lo + si
                    if l < NL - 1:
                        pt = ppool2.tile([64, 128], FP, tag="tp",
                                         name=f"tp{l}_{s}")
                        nc.tensor.transpose(pt[:], outg[:, si, :], ident[:])
                        xtn = xpool.tile([64, 128], FP, tag="xtn",
                                         name=f"xtn{l}_{s}")
                        nc.scalar.copy(xtn[:], pt[:])
                        matmul_block(l + 1, s, xtn[:], row[:], si)
                    else:
                        nc.scalar.copy(
                            row[:].bitcast(FP)[:, si, :], outg[:, si, :])
                emit_row_dma(l + 1, gi, row)

        # ---------------- final AllGathers + pooling ----------------
        pidx = cpool.tile([128, 8 * WPS], I16)
        nc.sync.dma_start(pidx[:], pidx_in[:])
        pmask = cpool.tile([128, WPS], FP)
        nc.sync.dma_start(pmask[:], pmask_in[:])
        pooled = cpool.tile([128, 64], FP)
        first = True
        for q in range(NQ):
            nc.gpsimd.collective_compute(
                "AllGather", ALU.bypass, ins=[slabq[NL][q][:].opt()],
                outs=[tabq[NL][q][:].opt()],
                replica_groups=[list(range(NCORES))])
        for q in range(NQ):
            wq = int(PWQS[q])
            c0 = int(pq0[q])
            PG = gpools[q].tile([128, wq, 128], I16, tag=f"G{q}",
                                name=f"PG_{q}")
            nc.gpsimd.dma_gather(
                PG[:], tabq[NL][q][:], pidx[:, 8 * c0: 8 * (c0 + wq)],
                128 * wq, 128 * wq, 128, single_packet=False, queue_num=q)
            PGf = PG[:].bitcast(FP)
            pm = wpool.tile([128, wq, 64], FP, tag="pm", name=f"pm_{q}")
            nc.vector.tensor_tensor(
                pm[:], PGf,
                pmask[:, c0:c0 + wq].unsqueeze(2).broadcast_to([128, wq, 64]),
                ALU.add)
            red = wpool.tile([128, 64], FP, tag="red", name=f"red_{q}")
            nc.vector.tensor_reduce(red[:], pm[:].rearrange("p w f -> p f w"),
                                    axis=AX.X, op=ALU.max)
            if first:
                nc.vector.tensor_copy(pooled[:], red[:])
                first = False
            else:
                nc.vector.tensor_tensor(pooled[:], pooled[:], red[:], ALU.max)
        ptp = ppool2.tile([64, 128], FP, tag="tp", name="pool_tp")
        nc.tensor.transpose(ptp[:], pooled[:], ident[:])
        ptps = cpool.tile([64, 128], FP)
        nc.scalar.copy(ptps[:], ptp[:])
        pooledT = cpool.tile([64, GPC], FP)
        nc.vector.tensor_reduce(
            pooledT[:], ptps[:].rearrange("p (g r) -> p g r", r=4),
            axis=AX.X, op=ALU.max)
        fcW = cpool.tile([64, 2], FP)
        nc.sync.dma_start(fcW[:], fcW_in[:])
        fcb = cpool.tile([GPC, 2], FP)
        nc.sync.dma_start(fcb[:], fcb_in[:])
        plog = ppool.tile([GPC, 2], FP, tag="mm", name="logits_mm")
        nc.tensor.matmul(plog[:], pooledT[:], fcW[:], start=True, stop=True)
        logits = cpool.tile([GPC, 2], FP)
        nc.vector.tensor_tensor(logits[:], plog[:], fcb[:], ALU.add)
        nc.sync.dma_start(logits_out[:], logits[:])
        m = cpool.tile([GPC, 1], FP)
        nc.vector.tensor_reduce(m[:], logits[:], axis=AX.X, op=ALU.max)
        z = cpool.tile([GPC, 2], FP)
        nc.vector.tensor_tensor(z[:], logits[:], m[:].broadcast_to([GPC, 2]),
                                ALU.subtract)
        ez = cpool.tile([GPC, 2], FP)
        nc.scalar.activation(ez[:], z[:], ACTF.Exp)
        den2 = cpool.tile([GPC, 1], FP)
        nc.vector.tensor_reduce(den2[:], ez[:], axis=AX.X, op=ALU.add)
        rden2 = cpool.tile([GPC, 1], FP)
        nc.vector.reciprocal(rden2[:], den2[:])
        probas = cpool.tile([GPC, 2], FP)
        nc.vector.tensor_tensor(probas[:], ez[:],
                                rden2[:].broadcast_to([GPC, 2]), ALU.mult)
        nc.sync.dma_start(probas_out[:], probas[:])
    return nc


# --------------------------------------------------------------- inputs
def make_inputs(P, inp):
    x = np.asarray(inp["x"], np.float32)
    wext_np = []
    for l in range(NL):
        Wl = np.asarray(inp[f"W{l+1}"], np.float32)
        As = expand_a(np.asarray(inp[f"a{l+1}s"], np.float32))
        Ad = expand_a(np.asarray(inp[f"a{l+1}d"], np.float32))
        wext_np.append(np.concatenate([Wl, Wl @ As, Wl @ Ad], axis=1))
    bias_np = np.stack([np.asarray(inp[f"b{l+1}"], np.float32)
                        for l in range(NL)])
    bias_rep = np.tile(bias_np[None], (128, 1, 1))
    fcW = np.asarray(inp["fcW"], np.float32)
    fcb = np.tile(np.asarray(inp["fcb"], np.float32)[None, :], (GPC, 1))

    NRNG = len(P['ranges'])
    chunk_base = P['chunk_base']
    chunk_cols = P['chunk_cols']

    def build_stream(lidx_c, regs):
        parts = []
        for c0, ncols in regs:
            stream = lidx_c[:, c0:c0 + ncols].T.reshape(1, -1)
            parts.append(wrap_idx(stream)[0])
        return np.concatenate(parts, axis=1).astype(np.int16)

    in_maps = []
    for c in range(NCORES):
        nodes = P["node_at"][c]
        xs = np.zeros((SLAB, 128), np.float32)
        valid = nodes >= 0
        xs[valid] = x[nodes[valid]]
        m = {
            "xT": np.ascontiguousarray(xs.T),
            "pool_idx": build_stream(
                P["pool_lidx"][c].astype(np.int64),
                [(int(P['pq0'][q]), int(P['PWQS'][q])) for q in range(NQ)]),
            "pool_maskneg": P["pool_maskneg"][c].astype(np.float32),
            "bias": bias_rep, "fcW": fcW, "fcb": fcb,
        }
        for q in range(NQ):
            regs = [(int(chunk_base[q][ri]), int(chunk_cols[q][ri]))
                    for ri in range(NRNG)]
            m[f"idx{q}"] = build_stream(P['lidx'][q][c], regs)
        for l in range(NL):
            m[f"wext{l}"] = wext_np[l]
        in_maps.append(m)
    return in_maps


def _run(inputs, trace=False, tmpdir=None):
    inp = {k: np.asarray(v) for k, v in inputs.items()}
    P = preprocess(inp['edge_index'], inp['batch'])
    in_maps = make_inputs(P, inp)
    nc = bacc.Bacc("TRN2", num_swdge_queues=4)
    build(nc, P)
    nc.compile()
    res = run_bass_kernel_spmd(nc, in_maps, list(range(NCORES)), trace=trace,
                               tmpdir=tmpdir)
    logits = np.zeros((256, 2), np.float32)
    probas = np.zeros((256, 2), np.float32)
    for c in range(NCORES):
        lg = res.results[c]["logits"]
        pb = res.results[c]["probas"]
        for r in range(GPC):
            g = P["out_graph"][c, r]
            logits[g] = lg[r]
            probas[g] = pb[r]
    return logits, probas, res.exec_time_ns


def kernel(**inputs):
    logits, probas, _ = _run(inputs, trace=False)
    return logits, probas
